# Initial kernel scaffold
#
"""Trainium2 Bass kernel for nn_Explainer (gnn_message_passing) — v2.

Math (reference):
  f12[i*n+j] = concat(embed[i], embed[j]);  h = relu(f12 @ W1 + b1)
  log_alpha = h @ W2 + b2
  gate = sigmoid((log(u) - log(1-u) + log_alpha) / beta)
  sym = (gate + gate.T)/2 ; masked = adj * sym
  hg = relu((masked @ x) @ Wg1); pooled = hg.mean(0); softmax(pooled @ Wg2)

Key decomposition (as v1): log_alpha[i,j] = W2 . relu(A[i] + B[j]) with
  A = embed @ W1[:64] + b1, B = embed @ W1[64:].

v2 structural change: NO ncfw collectives. Each core c (rows cb=c*128) computes
  H_c[h,i] = sum_{jl} xw[cb+jl,h] * adj[i,cb+jl] * gate[cb+jl,i]      (T2 part)
           + [i in cb] sum_j adj[i,j] * gate[i,j] * xw[j,h]           (T1 part)
  with xw = x @ (Wg1/2).  sum_c H_c = ((masked_adj @ x) @ Wg1/.. )^T == hg pre-relu.
The T1 part is placed at columns cb via a PE "scatter" matmul against a
runtime-built one-hot selection S_c[r,i] = (i == cb+r) (cb is an input).
The 8 partial H_c are summed with ONE 128KB fp16 AllReduce; a tiny
zero-dependency AllGather issued first absorbs the ~50us CC channel-setup
barrier under the compute.  Tail (relu/pool/logits/softmax) is computed
redundantly on every core; the harness reads core 0.
(The SWDGE remote_dma path would be ~40us faster but hangs under axon.)
"""
import numpy as np

import concourse.bass as bass
import concourse.bacc as bacc
import concourse.tile as tile
from concourse import mybir
from concourse.bass_utils import run_bass_kernel_spmd

N = 1024
NC = 8
R = N // NC          # 128 rows per core
D = 64               # embed dim
H = 64               # hidden
F = 128              # x features
C = 8                # classes
NPAIR = R // 2       # 64 i-pairs per core
GRP = 16             # pairs per PE column-group (32 cols / 2)

F32 = mybir.dt.float32
BF16 = mybir.dt.bfloat16
FP16 = mybir.dt.float16

MM_DT = BF16
DEBUG_OUTPUTS = False
EXCHANGE = "cc"      # "rdma" | "cc"  (rdma: SWDGE remote path — hangs under axon)


def _mask_w2_np():
    """[128, NPAIR, 32] mask: 1.0 where the block-diag W2 stack has W2 values."""
    cols = 32
    m = np.zeros((128, NPAIR, cols), np.float32)
    for t in range(NPAIR):
        s = t % GRP
        m[0:64, t, 2 * s] = 1.0
        m[64:128, t, 2 * s + 1] = 1.0
    return m


def build():
    nc = bacc.Bacc("TRN2", target_bir_lowering=False, debug=False, num_devices=NC)

    # ---- kernel I/O ----
    embT_in = nc.dram_tensor("embT_in", [D, N], BF16, kind="ExternalInput")
    embTs_in = nc.dram_tensor("embTs_in", [D, R], BF16, kind="ExternalInput")
    xT_in = nc.dram_tensor("xT_in", [F, N], BF16, kind="ExternalInput")
    xTcb_in = nc.dram_tensor("xTcb_in", [F, R], BF16, kind="ExternalInput")
    adjrow_in = nc.dram_tensor("adjrow_in", [R, N], BF16, kind="ExternalInput")
    adjcolT_in = nc.dram_tensor("adjcolT_in", [R, N], BF16, kind="ExternalInput")
    nlog_in = nc.dram_tensor("nlog_in", [R, N], FP16, kind="ExternalInput")
    scal_in = nc.dram_tensor("scal_in", [1, 3], F32, kind="ExternalInput")
    w1_in = nc.dram_tensor("w1_in", [2 * D, H], BF16, kind="ExternalInput")
    w2b1_in = nc.dram_tensor("w2b1_in", [H, 2], F32, kind="ExternalInput")
    wg1h_in = nc.dram_tensor("wg1h_in", [F, H], BF16, kind="ExternalInput")
    wg2s2_in = nc.dram_tensor("wg2s2_in", [2 * H, C], F32, kind="ExternalInput")
    out_dram = nc.dram_tensor("out", [1, C], F32, kind="ExternalOutput")

    dbg = {}
    if DEBUG_OUTPUTS:
        for nm, shp, dt in [("d_la", [R, N], F32), ("d_gate", [R, N], BF16),
                            ("d_m1", [R, N], BF16), ("d_m2", [R, N], BF16),
                            ("d_S", [R, N], BF16), ("d_T1", [R, H], BF16),
                            ("d_H", [128, 512], FP16),
                            ("d_accF", [128, 512], FP16),
                            ("d_xw", [128, NC * H], BF16)]:
            dbg[nm] = nc.dram_tensor(nm, shp, dt, kind="ExternalOutput")

    # ---- compile-time constants ----
    maskw2_c = nc.inline_tensor(
        _mask_w2_np().astype(mybir.dt.np(MM_DT)), name="maskw2")
    iota_rep_c = nc.inline_tensor(
        np.broadcast_to(np.arange(N, dtype=np.float32), (128, N)).copy(),
        name="iotarep")
    rcol_c = nc.inline_tensor(
        np.arange(R, dtype=np.float32).reshape(R, 1), name="rcol")

    # ---- cross-core exchange semaphores (same nums on all cores: SPMD) ----
    if EXCHANGE == "rdma":
        RS = [nc.alloc_semaphore(f"rs_{s}") for s in range(3)]
        LS = nc.alloc_semaphore("ls")
        PREP = nc.alloc_semaphore("prep")
        VD = nc.alloc_semaphore("vd")

    def rdests_for(delta):
        slots = [None] * 8
        slots[4 if delta == 4 else 0] = (0, delta)
        return slots

    with tile.TileContext(nc) as tc:
        with (
            tc.tile_pool(name="const", bufs=1) as constp,
            tc.tile_pool(name="big", bufs=1) as big,
            tc.tile_pool(name="tmpp", bufs=4) as tmpp,
            tc.tile_pool(name="pla", bufs=1, space="PSUM") as pla,
            tc.tile_pool(name="ptp", bufs=2, space="PSUM") as ptp,
            tc.tile_pool(name="psm", bufs=2, space="PSUM") as psm,
            tc.tile_pool(name="pH", bufs=1, space="PSUM") as pH,
            tc.tile_pool(name="dram", bufs=1, space="DRAM") as dram,
        ):
            # ================= phase 0: loads + precompute ==================
            # sync collective FIRST: zero-dependency trigger so the CC
            # channel-setup barrier runs concurrently with all compute.
            if EXCHANGE == "cc":
                sync_out = dram.tile([NC, 8], F32, addr_space="Shared")
                nc.gpsimd.collective_compute(
                    "AllGather", mybir.AluOpType.bypass,
                    replica_groups=[list(range(NC))],
                    ins=[iota_rep_c[0:1, 0:8].opt()], outs=[sync_out[:].opt()])

            # PE warm-up for the HAM clock gate.
            warm_sb = tmpp.tile([128, 512], MM_DT, tag="warm")
            nc.vector.memset(warm_sb[:], 0.0)
            for _ in range(10):
                warm_ps = pla.tile([1, 512], F32, tag="la0", name="warm_ps")
                nc.tensor.matmul(warm_ps[:], warm_sb[:, 0:1], warm_sb[:])

            # critical-path loads first (phase 1 prerequisites)
            w1a_sb = big.tile([D, H], BF16)
            nc.sync.dma_start(w1a_sb[:], w1_in[0:D, :])
            eTs = big.tile([D, R], BF16)
            nc.sync.dma_start(eTs[:], embTs_in[:])
            w2b1_sb = big.tile([H, 2], F32)
            nc.sync.dma_start(w2b1_sb[:], w2b1_in[:])
            embT = big.tile([D, N], BF16)
            nc.sync.dma_start(embT[:], embT_in[:])
            maskw2 = constp.tile([128, NPAIR, 32], MM_DT)
            nc.sync.dma_start(maskw2[:], maskw2_c[:])
            w1b_sb = big.tile([D, H], BF16)
            nc.scalar.dma_start(w1b_sb[:], w1_in[D:2 * D, :])

            # remaining loads spread across queues
            nlog_sb = big.tile([R, N], FP16)
            nc.scalar.dma_start(nlog_sb[:], nlog_in[:])
            adjrow = big.tile([R, N], BF16)
            nc.scalar.dma_start(adjrow[:], adjrow_in[:])
            adjcolT = big.tile([R, N], BF16)
            nc.scalar.dma_start(adjcolT[:], adjcolT_in[:])
            xT_sb = big.tile([F, N], BF16)
            nc.gpsimd.dma_start(xT_sb[:], xT_in[:])
            xTcb_sb = big.tile([F, R], BF16)
            nc.gpsimd.dma_start(xTcb_sb[:], xTcb_in[:])
            ones128 = constp.tile([1, 128], F32)
            nc.vector.memset(ones128[:], 1.0)
            iota_rep = big.tile([128, N], F32)
            nc.gpsimd.dma_start(iota_rep[:], iota_rep_c[:])
            rcol = constp.tile([R, 1], F32)
            nc.gpsimd.dma_start(rcol[:], rcol_c[:])
            scal_sb = big.tile([1, 3], F32)
            nc.gpsimd.dma_start(scal_sb[:], scal_in[:])
            cb_sb = scal_sb[:, 0:1]
            b2_sb = scal_sb[:, 1:2]
            tmp_sb = scal_sb[:, 2:3]
            wg1h_sb = big.tile([F, H], BF16)
            nc.scalar.dma_start(wg1h_sb[:], wg1h_in[:])
            wg2s2_sb = big.tile([2 * H, C], F32)
            nc.scalar.dma_start(wg2s2_sb[:], wg2s2_in[:])

            # A^T for this core's slab + ATstack
            at_ps = psm.tile([H, R], F32, tag="sm")
            nc.tensor.matmul(at_ps[:], w1a_sb[:], eTs[:])
            ats = big.tile([H, R], F32)
            nc.vector.tensor_scalar(out=ats[:], in0=at_ps[:],
                                    scalar1=w2b1_sb[:, 1:2], scalar2=None,
                                    op0=mybir.AluOpType.add)
            atstack = big.tile([128, NPAIR], F32)
            ats_pair = ats[:].rearrange("h (t two) -> h two t", two=2)
            nc.vector.tensor_copy(atstack[0:H, :], ats_pair[:, 0, :])
            nc.vector.tensor_copy(atstack[H:128, :], ats_pair[:, 1, :])

            # B^T (full) stacked twice -> [128, 1024] bf16
            btstack = big.tile([128, N], MM_DT)
            for jc in range(2):
                bt_ps = psm.tile([H, 512], F32, tag="sm")
                nc.tensor.matmul(bt_ps[:], w1b_sb[:],
                                 embT[:, jc * 512:(jc + 1) * 512])
                nc.vector.tensor_copy(
                    btstack[0:H, jc * 512:(jc + 1) * 512], bt_ps[:])
                nc.scalar.copy(
                    btstack[H:128, jc * 512:(jc + 1) * 512], bt_ps[:])

            # W2 stacks
            w2col = big.tile([128, 1], F32)
            nc.vector.tensor_copy(w2col[0:H, :], w2b1_sb[:, 0:1])
            nc.vector.tensor_copy(w2col[H:128, :], w2b1_sb[:, 0:1])
            w2s_t = big.tile([128, NPAIR, 32], MM_DT)
            nc.vector.tensor_scalar(
                out=w2s_t[:].rearrange("p t c -> p (t c)"),
                in0=maskw2[:].rearrange("p t c -> p (t c)"),
                scalar1=w2col[:], scalar2=None,
                op0=mybir.AluOpType.mult)

            # sigmoid scale/bias: sigmoid(invb * pre + invb*b2)
            invb = big.tile([1, 1], F32)
            nc.vector.reciprocal(invb[:], tmp_sb)
            ib2 = big.tile([1, 1], F32)
            nc.vector.tensor_tensor(ib2[:], invb[:], b2_sb,
                                    op=mybir.AluOpType.mult)
            invb_ps = psm.tile([128, 1], F32, tag="sm")
            nc.tensor.matmul(invb_ps[:], ones128[:], invb[:])
            invb128 = big.tile([128, 1], F32)
            nc.vector.tensor_copy(invb128[:], invb_ps[:])
            ib2_ps = psm.tile([128, 1], F32, tag="sm")
            nc.tensor.matmul(ib2_ps[:], ones128[:], ib2[:])
            ib2b = big.tile([128, 1], F32)
            nc.vector.tensor_copy(ib2b[:], ib2_ps[:])

            # ================= phase 1: edge MLP ============================
            la_ps = [pla.tile([128, 512], F32, tag=f"la{jc}", name=f"la_ps{jc}")
                     for jc in range(2)]
            for t in range(NPAIR):
                g, s = t // GRP, t % GRP
                tmpb = tmpp.tile([128, N], MM_DT, tag="relu")
                if t % 4 == 2 and t < 48:
                    nc.scalar.activation(
                        tmpb[:], btstack[:],
                        mybir.ActivationFunctionType.Relu,
                        bias=atstack[:, t:t + 1])
                else:
                    nc.vector.tensor_scalar(
                        out=tmpb[:], in0=btstack[:],
                        scalar1=atstack[:, t:t + 1], scalar2=0.0,
                        op0=mybir.AluOpType.add, op1=mybir.AluOpType.max)
                for jc in range(2):
                    nc.tensor.matmul(
                        la_ps[jc][32 * g:32 * (g + 1), :],
                        w2s_t[:, t, :],
                        tmpb[:, jc * 512:(jc + 1) * 512],
                        start=(s == 0), stop=(s == GRP - 1),
                        tile_position=(0, 32 * g))

            # phase-3 precompute on idle PE/ACT/DVE (cheap, before phase 2)
            cb_ps = psm.tile([128, 1], F32, tag="sm")
            nc.tensor.matmul(cb_ps[:], ones128[:], cb_sb)
            rcb = big.tile([R, 1], F32)
            nc.vector.tensor_tensor(rcb[:], cb_ps[0:R, :], rcol[:],
                                    op=mybir.AluOpType.add)
            identb = constp.tile([128, 128], BF16)
            nc.vector.tensor_scalar(out=identb[:], in0=iota_rep[:, 0:128],
                                    scalar1=rcol[:], scalar2=None,
                                    op0=mybir.AluOpType.is_equal)
            xw_sb = big.tile([128, NC, H], BF16)
            xwcb_sb = big.tile([128, H], BF16)
            xwcb_ps = psm.tile([128, H], F32, tag="sm")
            nc.tensor.matmul(xwcb_ps[:], xTcb_sb[:], wg1h_sb[:])
            nc.vector.tensor_copy(xwcb_sb[:], xwcb_ps[:])

            # ================= phase 2: concrete gate =======================
            gate = big.tile([R, N], BF16)
            m1 = big.tile([R, N], BF16)
            m2 = big.tile([R, N], BF16)
            for jc in range(2):
                sl = slice(jc * 512, (jc + 1) * 512)
                pre = tmpp.tile([R, 512], F32, tag="pre", name=f"pre{jc}")
                nc.vector.tensor_tensor(
                    pre[:], la_ps[jc][:], nlog_sb[:, sl],
                    op=mybir.AluOpType.add)
                nc.scalar.activation(gate[:, sl], pre[:],
                                     mybir.ActivationFunctionType.Sigmoid,
                                     bias=ib2b[:], scale=invb128[:])
                nc.vector.tensor_tensor(m1[:, sl], adjrow[:, sl], gate[:, sl],
                                        op=mybir.AluOpType.mult)
                nc.vector.tensor_tensor(m2[:, sl], adjcolT[:, sl], gate[:, sl],
                                        op=mybir.AluOpType.mult)

            for r in range(NC):
                xw_ps = psm.tile([128, H], F32, tag="sm")
                nc.tensor.matmul(xw_ps[:], xT_sb[:, r * 128:(r + 1) * 128],
                                 wg1h_sb[:])
                nc.scalar.copy(xw_sb[:, r, :], xw_ps[:])

            # T2-half MMs start the H accumulation as soon as m2 is ready
            H_ps = pH.tile([128, 512], F32, tag="Hps")
            for jc in range(2):
                sl = slice(jc * 512, (jc + 1) * 512)
                rows = slice(jc * 64, jc * 64 + 64)
                nc.tensor.matmul(H_ps[rows, :], xwcb_sb[:], m2[:, sl],
                                 start=True, stop=False,
                                 tile_position=(0, jc * 64))

            # m1T blocks via PE transpose
            m1T = big.tile([128, NC, 128], BF16)
            for r in range(NC):
                tp = ptp.tile([128, 128], BF16, tag="tp")
                nc.tensor.transpose(tp[:], m1[:, r * 128:(r + 1) * 128],
                                    identb[:])
                nc.vector.tensor_copy(m1T[:, r, :], tp[:])

            S_c = big.tile([R, N], BF16)
            nc.vector.tensor_scalar(out=S_c[:], in0=iota_rep[:],
                                    scalar1=rcb[:], scalar2=None,
                                    op0=mybir.AluOpType.is_equal)


            # T1_rh[r, h] = sum_j m1[r, j] xw[j, h]
            t1_ps = pH.tile([128, H], F32, tag="t1")
            for r in range(NC):
                nc.tensor.matmul(t1_ps[:], m1T[:, r, :], xw_sb[:, r, :],
                                 start=(r == 0), stop=(r == NC - 1))
            t1_sb = big.tile([128, H], BF16)
            nc.vector.tensor_copy(t1_sb[:], t1_ps[:])

            # T1-placement closes the H accumulation
            for jc in range(2):
                sl = slice(jc * 512, (jc + 1) * 512)
                rows = slice(jc * 64, jc * 64 + 64)
                tp_pos = (0, jc * 64)
                nc.tensor.matmul(H_ps[rows, :], t1_sb[:], S_c[:, sl],
                                 start=False, stop=True, tile_position=tp_pos)

            accs = [big.tile([128, 512], FP16, name=f"acc{i}") for i in range(4)]
            recvs = [big.tile([128, 512], FP16, name=f"recv{i}") for i in range(3)]
            nc.vector.tensor_copy(accs[0][:, 0:256], H_ps[:, 0:256])
            nc.scalar.copy(accs[0][:, 256:512], H_ps[:, 256:512])

            # ================= phase 4: XOR-tree AllReduce ==================
            if EXCHANGE == "rdma":
                with tc.tile_critical():
                    for s, dlt in enumerate([1, 2, 4]):
                        nc.gpsimd.remote_dma_broadcast(
                            recvs[s][:], accs[s][:], RS[s], LS,
                            rdests=rdests_for(dlt)).then_inc(PREP, 1)
                        nc.gpsimd.wait_ge(PREP, s + 1)
                        if s > 0:
                            nc.gpsimd.wait_ge(VD, s)
                        nc.gpsimd.trigger_dma(1)
                        nc.vector.wait_ge(RS[s], 2)
                        nc.vector.tensor_tensor(
                            accs[s + 1][:], accs[s][:], recvs[s][:],
                            op=mybir.AluOpType.add).then_inc(VD, 1)
                    nc.gpsimd.wait_ge(LS, 16 * 3)
                    for h in RS + [LS, PREP, VD]:
                        nc.gpsimd.sem_clear(h)
                accF = accs[3]
            else:
                ar_in = dram.tile([128, 512], FP16)
                nc.sync.dma_start(ar_in[:, 0:256], accs[0][:, 0:256])
                nc.scalar.dma_start(ar_in[:, 256:512], accs[0][:, 256:512])
                ar_out = dram.tile([128, 512], FP16, addr_space="Shared")
                nc.gpsimd.collective_compute(
                    "AllReduce", mybir.AluOpType.add,
                    replica_groups=[list(range(NC))],
                    ins=[ar_in[:].opt()], outs=[ar_out[:].opt()])
                accF = accs[3]
                nc.sync.dma_start(accF[:, 0:256], ar_out[:, 0:256])
                nc.scalar.dma_start(accF[:, 256:512], ar_out[:, 256:512])

            # ================= phase 5: tail ================================
            hgp = big.tile([128, 512], FP16)
            nc.vector.tensor_scalar(out=hgp[:], in0=accF[:], scalar1=0.0,
                                    scalar2=None, op0=mybir.AluOpType.max)
            red = big.tile([128, 1], F32)
            nc.vector.reduce_sum(red[:], hgp[:], axis=mybir.AxisListType.X)
            log_ps = psm.tile([1, C], F32, tag="sm")
            nc.tensor.matmul(log_ps[:], red[:], wg2s2_sb[:])
            e = big.tile([1, C], F32)
            nc.scalar.activation(e[:], log_ps[:],
                                 mybir.ActivationFunctionType.Exp)
            ssum = big.tile([1, 1], F32)
            nc.vector.reduce_sum(ssum[:], e[:], axis=mybir.AxisListType.X)
            rinv = big.tile([1, 1], F32)
            nc.vector.reciprocal(rinv[:], ssum[:])
            sm = big.tile([1, C], F32)
            nc.vector.tensor_scalar(out=sm[:], in0=e[:], scalar1=rinv[:],
                                    scalar2=None, op0=mybir.AluOpType.mult)
            nc.sync.dma_start(out_dram[:], sm[:])

            if DEBUG_OUTPUTS:
                la_dbg = big.tile([R, N], F32)
                for jc in range(2):
                    nc.scalar.copy(la_dbg[:, jc * 512:(jc + 1) * 512],
                                   la_ps[jc][:])
                nc.sync.dma_start(dbg["d_la"][:], la_dbg[:])
                nc.sync.dma_start(dbg["d_gate"][:], gate[:])
                nc.sync.dma_start(dbg["d_m1"][:], m1[:])
                nc.sync.dma_start(dbg["d_m2"][:], m2[:])
                nc.sync.dma_start(dbg["d_S"][:], S_c[:])
                nc.sync.dma_start(dbg["d_T1"][:], t1_sb[:, 0:H])
                nc.sync.dma_start(dbg["d_H"][:], accs[0][:])
                nc.sync.dma_start(dbg["d_accF"][:], accF[:])
                nc.sync.dma_start(
                    dbg["d_xw"][:],
                    xw_sb[:].rearrange("p r h -> p (r h)"))

    nc.compile()
    return nc


_NC_CACHE = None
_RUNNER_CACHE = None


def _get_nc():
    global _NC_CACHE
    if _NC_CACHE is None:
        _NC_CACHE = build()
    return _NC_CACHE


def _get_runner():
    """Cached jitted 8-core executable."""
    global _RUNNER_CACHE
    if _RUNNER_CACHE is not None:
        return _RUNNER_CACHE
    import jax
    from jax.sharding import Mesh, PartitionSpec
    from jax.experimental.shard_map import shard_map
    from concourse import mybir as mb
    from concourse.bass2jax import (_bass_exec_p, install_neuronx_cc_hook,
                                    partition_id_tensor)

    nc = _get_nc()
    install_neuronx_cc_hook()
    partition_name = (nc.partition_id_tensor.name
                      if nc.partition_id_tensor else None)
    in_names, out_names, out_avals, zero_outs = [], [], [], []
    for alloc in nc.m.functions[0].allocations:
        if not isinstance(alloc, mb.MemoryLocationSet):
            continue
        name = alloc.memorylocations[0].name
        if alloc.kind == "ExternalInput":
            if name == partition_name:
                continue
            in_names.append(name)
        elif alloc.kind == "ExternalOutput":
            shape = tuple(alloc.tensor_shape)
            dtype = mb.dt.np(alloc.dtype)
            out_names.append(name)
            out_avals.append(jax.core.ShapedArray(shape, dtype))
            zero_outs.append(np.zeros(shape, dtype))
    n_params = len(in_names)
    all_in = in_names + out_names
    if partition_name is not None:
        all_in = all_in + [partition_name]

    def _body(*args):
        operands = list(args)
        if partition_name is not None:
            operands.append(partition_id_tensor())
        outs = _bass_exec_p.bind(
            *operands,
            out_avals=tuple(out_avals),
            in_names=tuple(all_in),
            out_names=tuple(out_names),
            lowering_input_output_aliases=(),
            sim_require_finite=True,
            sim_require_nnan=True,
            nc=nc,
        )
        return tuple(outs)

    devices = jax.devices()[:NC]
    mesh = Mesh(np.asarray(devices), ("core",))
    n_outs = len(out_names)
    sharded = jax.jit(
        shard_map(_body, mesh=mesh,
                  in_specs=(PartitionSpec("core"),) * (n_params + n_outs),
                  out_specs=(PartitionSpec("core"),) * n_outs,
                  check_rep=False),
        donate_argnums=tuple(range(n_params, n_params + n_outs)),
        keep_unused=True)

    def run(in_maps):
        concat_in = [
            np.concatenate([np.asarray(in_maps[c][nm]) for c in range(NC)],
                           axis=0)
            for nm in in_names
        ]
        concat_zeros = [
            np.zeros((NC * z.shape[0], *z.shape[1:]), z.dtype)
            for z in zero_outs
        ]
        out_arrs = sharded(*concat_in, *concat_zeros)
        return [
            {nm: np.asarray(out_arrs[i]).reshape(NC, *out_avals[i].shape)[c]
             for i, nm in enumerate(out_names)}
            for c in range(NC)
        ]

    _RUNNER_CACHE = run
    return run


def kernel(**inputs):
    x = np.ascontiguousarray(np.asarray(inputs["x"], dtype=np.float32))
    embed = np.ascontiguousarray(np.asarray(inputs["embed"], dtype=np.float32))
    adj = np.ascontiguousarray(np.asarray(inputs["adj"], dtype=np.float32))
    tmp = np.asarray(inputs["tmp"], dtype=np.float32).reshape(1, 1)
    noise = np.asarray(inputs["noise"], dtype=np.float32).reshape(N, N)
    W1 = np.ascontiguousarray(np.asarray(inputs["W1"], dtype=np.float32))
    b1 = np.asarray(inputs["b1"], dtype=np.float32).reshape(1, H)
    W2 = np.ascontiguousarray(np.asarray(inputs["W2"], dtype=np.float32))
    b2 = np.asarray(inputs["b2"], dtype=np.float32).reshape(1, 1)
    Wg1 = np.ascontiguousarray(np.asarray(inputs["Wg1"], dtype=np.float32))
    Wg2 = np.ascontiguousarray(np.asarray(inputs["Wg2"], dtype=np.float32))

    in_maps = build_in_maps(x, embed, adj, noise, tmp, W1, b1, W2, b2, Wg1, Wg2)
    try:
        results = _get_runner()(in_maps)
        return np.asarray(results[0]["out"], dtype=np.float32).reshape(1, C)
    except Exception:
        nc = _get_nc()
        res = run_bass_kernel_spmd(nc, in_maps, core_ids=list(range(NC)))
        return np.asarray(res.results[0]["out"],
                          dtype=np.float32).reshape(1, C)


def build_in_maps(x, embed, adj, noise, tmp, W1, b1, W2, b2, Wg1, Wg2):
    bf = mybir.dt.np(BF16)
    f16 = np.float16
    embT = np.ascontiguousarray(embed.T).astype(bf)
    xT = np.ascontiguousarray(x.T).astype(bf)
    nlog = (np.log(noise) - np.log1p(-noise)).astype(np.float32)
    wg1h = (0.5 * Wg1).astype(np.float32)
    wg2s = (Wg2 / float(N)).astype(np.float32)
    wg2s2 = np.ascontiguousarray(np.vstack([wg2s, wg2s]))
    in_maps = []
    for c in range(NC):
        sl = slice(c * R, (c + 1) * R)
        in_maps.append({
            "embT_in": embT,
            "embTs_in": np.ascontiguousarray(embT[:, sl]),
            "xT_in": xT,
            "xTcb_in": np.ascontiguousarray(xT[:, sl]),
            "adjrow_in": np.ascontiguousarray(adj[sl]).astype(bf),
            "adjcolT_in": np.ascontiguousarray(adj[:, sl].T).astype(bf),
            "nlog_in": np.ascontiguousarray(nlog[sl]).astype(f16),
            "scal_in": np.array([[c * R, b2[0, 0], tmp[0, 0]]],
                                dtype=np.float32),
            "w1_in": W1.astype(bf),
            "w2b1_in": np.ascontiguousarray(
                np.hstack([W2.reshape(H, 1), b1.reshape(H, 1)])),
            "wg1h_in": wg1h.astype(bf),
            "wg2s2_in": wg2s2,
        })
    return in_maps



# revision 1
# speedup vs baseline: 2.8430x; 2.8430x over previous
"""Trainium2 Bass kernel for nn_Explainer (gnn_message_passing) — v2.

Math (reference):
  f12[i*n+j] = concat(embed[i], embed[j]);  h = relu(f12 @ W1 + b1)
  log_alpha = h @ W2 + b2
  gate = sigmoid((log(u) - log(1-u) + log_alpha) / beta)
  sym = (gate + gate.T)/2 ; masked = adj * sym
  hg = relu((masked @ x) @ Wg1); pooled = hg.mean(0); softmax(pooled @ Wg2)

Key decomposition (as v1): log_alpha[i,j] = W2 . relu(A[i] + B[j]) with
  A = embed @ W1[:64] + b1, B = embed @ W1[64:].

v2 structural change: NO ncfw collectives. Each core c (rows cb=c*128) computes
  H_c[h,i] = sum_{jl} xw[cb+jl,h] * adj[i,cb+jl] * gate[cb+jl,i]      (T2 part)
           + [i in cb] sum_j adj[i,j] * gate[i,j] * xw[j,h]           (T1 part)
  with xw = x @ (Wg1/2).  sum_c H_c = ((masked_adj @ x) @ Wg1/.. )^T == hg pre-relu.
The T1 part is placed at columns cb via a PE "scatter" matmul against a
runtime-built one-hot selection S_c[r,i] = (i == cb+r) (cb is an input).
The 8 partial H_c are summed with ONE 128KB fp16 AllReduce; a tiny
zero-dependency AllGather issued first absorbs the ~50us CC channel-setup
barrier under the compute.  Tail (relu/pool/logits/softmax) is computed
redundantly on every core; the harness reads core 0.
(The SWDGE remote_dma path would be ~40us faster but hangs under axon.)
"""
import numpy as np

import concourse.bass as bass
import concourse.bacc as bacc
import concourse.tile as tile
from concourse import mybir
from concourse.bass_utils import run_bass_kernel_spmd

N = 1024
NC = 8
R = N // NC          # 128 rows per core
D = 64               # embed dim
H = 64               # hidden
F = 128              # x features
C = 8                # classes
NPAIR = R // 2       # 64 i-pairs per core
GRP = 16             # pairs per PE column-group (32 cols / 2)

F32 = mybir.dt.float32
BF16 = mybir.dt.bfloat16
FP16 = mybir.dt.float16

MM_DT = BF16
DEBUG_OUTPUTS = False
EXCHANGE = "cc"      # "rdma" | "cc"  (rdma: SWDGE remote path — hangs under axon)


def _mask_w2_np():
    """[128, NPAIR, 32] mask: 1.0 where the block-diag W2 stack has W2 values."""
    cols = 32
    m = np.zeros((128, NPAIR, cols), np.float32)
    for t in range(NPAIR):
        s = t % GRP
        m[0:64, t, 2 * s] = 1.0
        m[64:128, t, 2 * s + 1] = 1.0
    return m


def build():
    nc = bacc.Bacc("TRN2", target_bir_lowering=False, debug=False, num_devices=NC)

    # ---- kernel I/O ----
    embT_in = nc.dram_tensor("embT_in", [D, N], BF16, kind="ExternalInput")
    embTs_in = nc.dram_tensor("embTs_in", [D, R], BF16, kind="ExternalInput")
    xT_in = nc.dram_tensor("xT_in", [F, N], BF16, kind="ExternalInput")
    xTcb_in = nc.dram_tensor("xTcb_in", [F, R], BF16, kind="ExternalInput")
    adjrow_in = nc.dram_tensor("adjrow_in", [R, N], BF16, kind="ExternalInput")
    adjcolT_in = nc.dram_tensor("adjcolT_in", [R, N], BF16, kind="ExternalInput")
    nlog_in = nc.dram_tensor("nlog_in", [R, N], FP16, kind="ExternalInput")
    scal_in = nc.dram_tensor("scal_in", [1, 3], F32, kind="ExternalInput")
    w1_in = nc.dram_tensor("w1_in", [2 * D, H], BF16, kind="ExternalInput")
    w2b1_in = nc.dram_tensor("w2b1_in", [H, 2], F32, kind="ExternalInput")
    wg1h_in = nc.dram_tensor("wg1h_in", [F, H], BF16, kind="ExternalInput")
    wg2s2_in = nc.dram_tensor("wg2s2_in", [2 * H, C], F32, kind="ExternalInput")
    out_dram = nc.dram_tensor("out", [1, C], F32, kind="ExternalOutput")

    dbg = {}
    if DEBUG_OUTPUTS:
        for nm, shp, dt in [("d_la", [R, N], F32), ("d_gate", [R, N], BF16),
                            ("d_m1", [R, N], BF16), ("d_m2", [R, N], BF16),
                            ("d_S", [R, N], BF16), ("d_T1", [R, H], BF16),
                            ("d_H", [128, 512], FP16),
                            ("d_accF", [128, 512], FP16),
                            ("d_xw", [128, NC * H], BF16)]:
            dbg[nm] = nc.dram_tensor(nm, shp, dt, kind="ExternalOutput")

    # ---- compile-time constants ----
    maskw2_c = nc.inline_tensor(
        _mask_w2_np().astype(mybir.dt.np(MM_DT)), name="maskw2")
    iota_rep_c = nc.inline_tensor(
        np.broadcast_to(np.arange(N, dtype=np.float32), (128, N)).copy(),
        name="iotarep")
    rcol_c = nc.inline_tensor(
        np.arange(R, dtype=np.float32).reshape(R, 1), name="rcol")

    # ---- cross-core exchange semaphores (same nums on all cores: SPMD) ----
    if EXCHANGE == "rdma":
        RS = [nc.alloc_semaphore(f"rs_{s}") for s in range(3)]
        LS = nc.alloc_semaphore("ls")
        PREP = nc.alloc_semaphore("prep")
        VD = nc.alloc_semaphore("vd")

    def rdests_for(delta):
        slots = [None] * 8
        slots[4 if delta == 4 else 0] = (0, delta)
        return slots

    with tile.TileContext(nc) as tc:
        with (
            tc.tile_pool(name="const", bufs=1) as constp,
            tc.tile_pool(name="big", bufs=1) as big,
            tc.tile_pool(name="tmpp", bufs=4) as tmpp,
            tc.tile_pool(name="pla", bufs=1, space="PSUM") as pla,
            tc.tile_pool(name="ptp", bufs=2, space="PSUM") as ptp,
            tc.tile_pool(name="psm", bufs=2, space="PSUM") as psm,
            tc.tile_pool(name="pH", bufs=1, space="PSUM") as pH,
            tc.tile_pool(name="dram", bufs=1, space="DRAM") as dram,
        ):
            # ================= phase 0: loads + precompute ==================
            # sync collective FIRST: zero-dependency trigger so the CC
            # channel-setup barrier runs concurrently with all compute.
            if EXCHANGE == "cc":
                sync_out = dram.tile([NC, 8], F32, addr_space="Shared")
                nc.gpsimd.collective_compute(
                    "AllGather", mybir.AluOpType.bypass,
                    replica_groups=[list(range(NC))],
                    ins=[iota_rep_c[0:1, 0:8].opt()], outs=[sync_out[:].opt()])

            # PE warm-up for the HAM clock gate.
            warm_sb = tmpp.tile([128, 512], MM_DT, tag="warm")
            nc.vector.memset(warm_sb[:], 0.0)
            for _ in range(10):
                warm_ps = pla.tile([1, 512], F32, tag="la0", name="warm_ps")
                nc.tensor.matmul(warm_ps[:], warm_sb[:, 0:1], warm_sb[:])

            # critical-path loads first (phase 1 prerequisites)
            w1a_sb = big.tile([D, H], BF16)
            nc.sync.dma_start(w1a_sb[:], w1_in[0:D, :])
            eTs = big.tile([D, R], BF16)
            nc.sync.dma_start(eTs[:], embTs_in[:])
            w2b1_sb = big.tile([H, 2], F32)
            nc.sync.dma_start(w2b1_sb[:], w2b1_in[:])
            embT = big.tile([D, N], BF16)
            nc.sync.dma_start(embT[:], embT_in[:])
            maskw2 = constp.tile([128, NPAIR, 32], MM_DT)
            nc.sync.dma_start(maskw2[:], maskw2_c[:])
            w1b_sb = big.tile([D, H], BF16)
            nc.scalar.dma_start(w1b_sb[:], w1_in[D:2 * D, :])

            # remaining loads spread across queues
            nlog_sb = big.tile([R, N], FP16)
            nc.scalar.dma_start(nlog_sb[:], nlog_in[:])
            adjrow = big.tile([R, N], BF16)
            nc.scalar.dma_start(adjrow[:], adjrow_in[:])
            adjcolT = big.tile([R, N], BF16)
            nc.scalar.dma_start(adjcolT[:], adjcolT_in[:])
            xT_sb = big.tile([F, N], BF16)
            nc.gpsimd.dma_start(xT_sb[:], xT_in[:])
            xTcb_sb = big.tile([F, R], BF16)
            nc.gpsimd.dma_start(xTcb_sb[:], xTcb_in[:])
            ones128 = constp.tile([1, 128], F32)
            nc.vector.memset(ones128[:], 1.0)
            iota_rep = big.tile([128, N], F32)
            nc.gpsimd.dma_start(iota_rep[:], iota_rep_c[:])
            rcol = constp.tile([R, 1], F32)
            nc.gpsimd.dma_start(rcol[:], rcol_c[:])
            scal_sb = big.tile([1, 3], F32)
            nc.gpsimd.dma_start(scal_sb[:], scal_in[:])
            cb_sb = scal_sb[:, 0:1]
            b2_sb = scal_sb[:, 1:2]
            tmp_sb = scal_sb[:, 2:3]
            wg1h_sb = big.tile([F, H], BF16)
            nc.scalar.dma_start(wg1h_sb[:], wg1h_in[:])
            wg2s2_sb = big.tile([2 * H, C], F32)
            nc.scalar.dma_start(wg2s2_sb[:], wg2s2_in[:])

            # A^T for this core's slab + ATstack
            at_ps = psm.tile([H, R], F32, tag="sm")
            nc.tensor.matmul(at_ps[:], w1a_sb[:], eTs[:])
            ats = big.tile([H, R], F32)
            nc.vector.tensor_scalar(out=ats[:], in0=at_ps[:],
                                    scalar1=w2b1_sb[:, 1:2], scalar2=None,
                                    op0=mybir.AluOpType.add)
            atstack = big.tile([128, NPAIR], F32)
            ats_pair = ats[:].rearrange("h (t two) -> h two t", two=2)
            nc.vector.tensor_copy(atstack[0:H, :], ats_pair[:, 0, :])
            nc.vector.tensor_copy(atstack[H:128, :], ats_pair[:, 1, :])

            # B^T (full) stacked twice -> [128, 1024] bf16
            btstack = big.tile([128, N], MM_DT)
            for jc in range(2):
                bt_ps = psm.tile([H, 512], F32, tag="sm")
                nc.tensor.matmul(bt_ps[:], w1b_sb[:],
                                 embT[:, jc * 512:(jc + 1) * 512])
                nc.vector.tensor_copy(
                    btstack[0:H, jc * 512:(jc + 1) * 512], bt_ps[:])
                nc.scalar.copy(
                    btstack[H:128, jc * 512:(jc + 1) * 512], bt_ps[:])

            # W2 stacks
            w2col = big.tile([128, 1], F32)
            nc.vector.tensor_copy(w2col[0:H, :], w2b1_sb[:, 0:1])
            nc.vector.tensor_copy(w2col[H:128, :], w2b1_sb[:, 0:1])
            w2s_t = big.tile([128, NPAIR, 32], MM_DT)
            nc.vector.tensor_scalar(
                out=w2s_t[:].rearrange("p t c -> p (t c)"),
                in0=maskw2[:].rearrange("p t c -> p (t c)"),
                scalar1=w2col[:], scalar2=None,
                op0=mybir.AluOpType.mult)

            # sigmoid scale/bias: sigmoid(invb * pre + invb*b2)
            invb = big.tile([1, 1], F32)
            nc.vector.reciprocal(invb[:], tmp_sb)
            ib2 = big.tile([1, 1], F32)
            nc.vector.tensor_tensor(ib2[:], invb[:], b2_sb,
                                    op=mybir.AluOpType.mult)
            invb_ps = psm.tile([128, 1], F32, tag="sm")
            nc.tensor.matmul(invb_ps[:], ones128[:], invb[:])
            invb128 = big.tile([128, 1], F32)
            nc.vector.tensor_copy(invb128[:], invb_ps[:])
            ib2_ps = psm.tile([128, 1], F32, tag="sm")
            nc.tensor.matmul(ib2_ps[:], ones128[:], ib2[:])
            ib2b = big.tile([128, 1], F32)
            nc.vector.tensor_copy(ib2b[:], ib2_ps[:])

            # ================= phase 1: edge MLP ============================
            la_ps = [pla.tile([128, 512], F32, tag=f"la{jc}", name=f"la_ps{jc}")
                     for jc in range(2)]
            for t in range(NPAIR):
                g, s = t // GRP, t % GRP
                tmpb = tmpp.tile([128, N], MM_DT, tag="relu")
                if t % 4 == 2 and t < 48:
                    nc.scalar.activation(
                        tmpb[:], btstack[:],
                        mybir.ActivationFunctionType.Relu,
                        bias=atstack[:, t:t + 1])
                else:
                    nc.vector.tensor_scalar(
                        out=tmpb[:], in0=btstack[:],
                        scalar1=atstack[:, t:t + 1], scalar2=0.0,
                        op0=mybir.AluOpType.add, op1=mybir.AluOpType.max)
                for jc in range(2):
                    nc.tensor.matmul(
                        la_ps[jc][32 * g:32 * (g + 1), :],
                        w2s_t[:, t, :],
                        tmpb[:, jc * 512:(jc + 1) * 512],
                        start=(s == 0), stop=(s == GRP - 1),
                        tile_position=(0, 32 * g))

            # phase-3 precompute on idle PE/ACT/DVE (cheap, before phase 2)
            cb_ps = psm.tile([128, 1], F32, tag="sm")
            nc.tensor.matmul(cb_ps[:], ones128[:], cb_sb)
            rcb = big.tile([R, 1], F32)
            nc.vector.tensor_tensor(rcb[:], cb_ps[0:R, :], rcol[:],
                                    op=mybir.AluOpType.add)
            identb = constp.tile([128, 128], BF16)
            nc.vector.tensor_scalar(out=identb[:], in0=iota_rep[:, 0:128],
                                    scalar1=rcol[:], scalar2=None,
                                    op0=mybir.AluOpType.is_equal)
            xw_sb = big.tile([128, NC, H], BF16)
            xwcb_sb = big.tile([128, H], BF16)
            xwcb_ps = psm.tile([128, H], F32, tag="sm")
            nc.tensor.matmul(xwcb_ps[:], xTcb_sb[:], wg1h_sb[:])
            nc.vector.tensor_copy(xwcb_sb[:], xwcb_ps[:])

            # ================= phase 2: concrete gate =======================
            gate = big.tile([R, N], BF16)
            m1 = big.tile([R, N], BF16)
            m2 = big.tile([R, N], BF16)
            for jc in range(2):
                sl = slice(jc * 512, (jc + 1) * 512)
                pre = tmpp.tile([R, 512], F32, tag="pre", name=f"pre{jc}")
                nc.vector.tensor_tensor(
                    pre[:], la_ps[jc][:], nlog_sb[:, sl],
                    op=mybir.AluOpType.add)
                nc.scalar.activation(gate[:, sl], pre[:],
                                     mybir.ActivationFunctionType.Sigmoid,
                                     bias=ib2b[:], scale=invb128[:])
                nc.vector.tensor_tensor(m1[:, sl], adjrow[:, sl], gate[:, sl],
                                        op=mybir.AluOpType.mult)
                nc.vector.tensor_tensor(m2[:, sl], adjcolT[:, sl], gate[:, sl],
                                        op=mybir.AluOpType.mult)

            for r in range(NC):
                xw_ps = psm.tile([128, H], F32, tag="sm")
                nc.tensor.matmul(xw_ps[:], xT_sb[:, r * 128:(r + 1) * 128],
                                 wg1h_sb[:])
                nc.scalar.copy(xw_sb[:, r, :], xw_ps[:])

            # T2-half MMs start the H accumulation as soon as m2 is ready
            H_ps = pH.tile([128, 512], F32, tag="Hps")
            for jc in range(2):
                sl = slice(jc * 512, (jc + 1) * 512)
                rows = slice(jc * 64, jc * 64 + 64)
                nc.tensor.matmul(H_ps[rows, :], xwcb_sb[:], m2[:, sl],
                                 start=True, stop=False,
                                 tile_position=(0, jc * 64))

            # m1T blocks via PE transpose
            m1T = big.tile([128, NC, 128], BF16)
            for r in range(NC):
                tp = ptp.tile([128, 128], BF16, tag="tp")
                nc.tensor.transpose(tp[:], m1[:, r * 128:(r + 1) * 128],
                                    identb[:])
                nc.vector.tensor_copy(m1T[:, r, :], tp[:])

            S_c = big.tile([R, N], BF16)
            nc.vector.tensor_scalar(out=S_c[:], in0=iota_rep[:],
                                    scalar1=rcb[:], scalar2=None,
                                    op0=mybir.AluOpType.is_equal)


            # T1_rh[r, h] = sum_j m1[r, j] xw[j, h]
            t1_ps = pH.tile([128, H], F32, tag="t1")
            for r in range(NC):
                nc.tensor.matmul(t1_ps[:], m1T[:, r, :], xw_sb[:, r, :],
                                 start=(r == 0), stop=(r == NC - 1))
            t1_sb = big.tile([128, H], BF16)
            nc.vector.tensor_copy(t1_sb[:], t1_ps[:])

            # T1-placement closes the H accumulation
            for jc in range(2):
                sl = slice(jc * 512, (jc + 1) * 512)
                rows = slice(jc * 64, jc * 64 + 64)
                tp_pos = (0, jc * 64)
                nc.tensor.matmul(H_ps[rows, :], t1_sb[:], S_c[:, sl],
                                 start=False, stop=True, tile_position=tp_pos)

            accs = [big.tile([128, 512], FP16, name=f"acc{i}") for i in range(4)]
            recvs = [big.tile([128, 512], FP16, name=f"recv{i}") for i in range(3)]
            nc.vector.tensor_copy(accs[0][:, 0:256], H_ps[:, 0:256])
            nc.scalar.copy(accs[0][:, 256:512], H_ps[:, 256:512])

            # ================= phase 4: XOR-tree AllReduce ==================
            if EXCHANGE == "rdma":
                with tc.tile_critical():
                    for s, dlt in enumerate([1, 2, 4]):
                        nc.gpsimd.remote_dma_broadcast(
                            recvs[s][:], accs[s][:], RS[s], LS,
                            rdests=rdests_for(dlt)).then_inc(PREP, 1)
                        nc.gpsimd.wait_ge(PREP, s + 1)
                        if s > 0:
                            nc.gpsimd.wait_ge(VD, s)
                        nc.gpsimd.trigger_dma(1)
                        nc.vector.wait_ge(RS[s], 2)
                        nc.vector.tensor_tensor(
                            accs[s + 1][:], accs[s][:], recvs[s][:],
                            op=mybir.AluOpType.add).then_inc(VD, 1)
                    nc.gpsimd.wait_ge(LS, 16 * 3)
                    for h in RS + [LS, PREP, VD]:
                        nc.gpsimd.sem_clear(h)
                accF = accs[3]
            else:
                ar_in = dram.tile([128, 512], FP16)
                nc.sync.dma_start(ar_in[:, 0:256], accs[0][:, 0:256])
                nc.scalar.dma_start(ar_in[:, 256:512], accs[0][:, 256:512])
                ar_out = dram.tile([128, 512], FP16, addr_space="Shared")
                nc.gpsimd.collective_compute(
                    "AllReduce", mybir.AluOpType.add,
                    replica_groups=[list(range(NC))],
                    ins=[ar_in[:].opt()], outs=[ar_out[:].opt()])
                accF = accs[3]
                nc.sync.dma_start(accF[:, 0:256], ar_out[:, 0:256])
                nc.scalar.dma_start(accF[:, 256:512], ar_out[:, 256:512])

            # ================= phase 5: tail ================================
            hgp = big.tile([128, 512], FP16)
            nc.vector.tensor_scalar(out=hgp[:], in0=accF[:], scalar1=0.0,
                                    scalar2=None, op0=mybir.AluOpType.max)
            red = big.tile([128, 1], F32)
            nc.vector.reduce_sum(red[:], hgp[:], axis=mybir.AxisListType.X)
            log_ps = psm.tile([1, C], F32, tag="sm")
            nc.tensor.matmul(log_ps[:], red[:], wg2s2_sb[:])
            e = big.tile([1, C], F32)
            nc.scalar.activation(e[:], log_ps[:],
                                 mybir.ActivationFunctionType.Exp)
            ssum = big.tile([1, 1], F32)
            nc.vector.reduce_sum(ssum[:], e[:], axis=mybir.AxisListType.X)
            rinv = big.tile([1, 1], F32)
            nc.vector.reciprocal(rinv[:], ssum[:])
            sm = big.tile([1, C], F32)
            nc.vector.tensor_scalar(out=sm[:], in0=e[:], scalar1=rinv[:],
                                    scalar2=None, op0=mybir.AluOpType.mult)
            nc.sync.dma_start(out_dram[:], sm[:])

            if DEBUG_OUTPUTS:
                la_dbg = big.tile([R, N], F32)
                for jc in range(2):
                    nc.scalar.copy(la_dbg[:, jc * 512:(jc + 1) * 512],
                                   la_ps[jc][:])
                nc.sync.dma_start(dbg["d_la"][:], la_dbg[:])
                nc.sync.dma_start(dbg["d_gate"][:], gate[:])
                nc.sync.dma_start(dbg["d_m1"][:], m1[:])
                nc.sync.dma_start(dbg["d_m2"][:], m2[:])
                nc.sync.dma_start(dbg["d_S"][:], S_c[:])
                nc.sync.dma_start(dbg["d_T1"][:], t1_sb[:, 0:H])
                nc.sync.dma_start(dbg["d_H"][:], accs[0][:])
                nc.sync.dma_start(dbg["d_accF"][:], accF[:])
                nc.sync.dma_start(
                    dbg["d_xw"][:],
                    xw_sb[:].rearrange("p r h -> p (r h)"))

    nc.compile()
    return nc


_NC_CACHE = None
_RUNNER_CACHE = None


def _get_nc():
    global _NC_CACHE
    if _NC_CACHE is None:
        _NC_CACHE = build()
    return _NC_CACHE


def _get_runner():
    """Cached jitted 8-core executable."""
    global _RUNNER_CACHE
    if _RUNNER_CACHE is not None:
        return _RUNNER_CACHE
    import jax
    from jax.sharding import Mesh, PartitionSpec
    from jax.experimental.shard_map import shard_map
    from concourse import mybir as mb
    from concourse.bass2jax import (_bass_exec_p, install_neuronx_cc_hook,
                                    partition_id_tensor)

    nc = _get_nc()
    install_neuronx_cc_hook()
    partition_name = (nc.partition_id_tensor.name
                      if nc.partition_id_tensor else None)
    in_names, out_names, out_avals, zero_outs = [], [], [], []
    for alloc in nc.m.functions[0].allocations:
        if not isinstance(alloc, mb.MemoryLocationSet):
            continue
        name = alloc.memorylocations[0].name
        if alloc.kind == "ExternalInput":
            if name == partition_name:
                continue
            in_names.append(name)
        elif alloc.kind == "ExternalOutput":
            shape = tuple(alloc.tensor_shape)
            dtype = mb.dt.np(alloc.dtype)
            out_names.append(name)
            out_avals.append(jax.core.ShapedArray(shape, dtype))
            zero_outs.append(np.zeros(shape, dtype))
    n_params = len(in_names)
    all_in = in_names + out_names
    if partition_name is not None:
        all_in = all_in + [partition_name]

    def _body(*args):
        operands = list(args)
        if partition_name is not None:
            operands.append(partition_id_tensor())
        outs = _bass_exec_p.bind(
            *operands,
            out_avals=tuple(out_avals),
            in_names=tuple(all_in),
            out_names=tuple(out_names),
            lowering_input_output_aliases=(),
            sim_require_finite=True,
            sim_require_nnan=True,
            nc=nc,
        )
        return tuple(outs)

    devices = jax.devices()[:NC]
    mesh = Mesh(np.asarray(devices), ("core",))
    n_outs = len(out_names)
    sharded = jax.jit(
        shard_map(_body, mesh=mesh,
                  in_specs=(PartitionSpec("core"),) * (n_params + n_outs),
                  out_specs=(PartitionSpec("core"),) * n_outs,
                  check_rep=False),
        donate_argnums=tuple(range(n_params, n_params + n_outs)),
        keep_unused=True)

    def run(in_maps):
        concat_in = [
            np.concatenate([np.asarray(in_maps[c][nm]) for c in range(NC)],
                           axis=0)
            for nm in in_names
        ]
        concat_zeros = [
            np.zeros((NC * z.shape[0], *z.shape[1:]), z.dtype)
            for z in zero_outs
        ]
        out_arrs = sharded(*concat_in, *concat_zeros)
        return [
            {nm: np.asarray(out_arrs[i]).reshape(NC, *out_avals[i].shape)[c]
             for i, nm in enumerate(out_names)}
            for c in range(NC)
        ]

    _RUNNER_CACHE = run
    return run


def kernel(**inputs):
    x = np.ascontiguousarray(np.asarray(inputs["x"], dtype=np.float32))
    embed = np.ascontiguousarray(np.asarray(inputs["embed"], dtype=np.float32))
    adj = np.ascontiguousarray(np.asarray(inputs["adj"], dtype=np.float32))
    tmp = np.asarray(inputs["tmp"], dtype=np.float32).reshape(1, 1)
    noise = np.asarray(inputs["noise"], dtype=np.float32).reshape(N, N)
    W1 = np.ascontiguousarray(np.asarray(inputs["W1"], dtype=np.float32))
    b1 = np.asarray(inputs["b1"], dtype=np.float32).reshape(1, H)
    W2 = np.ascontiguousarray(np.asarray(inputs["W2"], dtype=np.float32))
    b2 = np.asarray(inputs["b2"], dtype=np.float32).reshape(1, 1)
    Wg1 = np.ascontiguousarray(np.asarray(inputs["Wg1"], dtype=np.float32))
    Wg2 = np.ascontiguousarray(np.asarray(inputs["Wg2"], dtype=np.float32))

    in_maps = build_in_maps(x, embed, adj, noise, tmp, W1, b1, W2, b2, Wg1, Wg2)
    try:
        results = _get_runner()(in_maps)
        return np.asarray(results[0]["out"], dtype=np.float32).reshape(1, C)
    except Exception:
        nc = _get_nc()
        res = run_bass_kernel_spmd(nc, in_maps, core_ids=list(range(NC)))
        return np.asarray(res.results[0]["out"],
                          dtype=np.float32).reshape(1, C)


def build_in_maps(x, embed, adj, noise, tmp, W1, b1, W2, b2, Wg1, Wg2):
    bf = mybir.dt.np(BF16)
    f16 = np.float16
    embT = np.ascontiguousarray(embed.T).astype(bf)
    xT = np.ascontiguousarray(x.T).astype(bf)
    nlog = (np.log(noise) - np.log1p(-noise)).astype(np.float32)
    wg1h = (0.5 * Wg1).astype(np.float32)
    wg2s = (Wg2 / float(N)).astype(np.float32)
    wg2s2 = np.ascontiguousarray(np.vstack([wg2s, wg2s]))
    in_maps = []
    for c in range(NC):
        sl = slice(c * R, (c + 1) * R)
        in_maps.append({
            "embT_in": embT,
            "embTs_in": np.ascontiguousarray(embT[:, sl]),
            "xT_in": xT,
            "xTcb_in": np.ascontiguousarray(xT[:, sl]),
            "adjrow_in": np.ascontiguousarray(adj[sl]).astype(bf),
            "adjcolT_in": np.ascontiguousarray(adj[:, sl].T).astype(bf),
            "nlog_in": np.ascontiguousarray(nlog[sl]).astype(f16),
            "scal_in": np.array([[c * R, b2[0, 0], tmp[0, 0]]],
                                dtype=np.float32),
            "w1_in": W1.astype(bf),
            "w2b1_in": np.ascontiguousarray(
                np.hstack([W2.reshape(H, 1), b1.reshape(H, 1)])),
            "wg1h_in": wg1h.astype(bf),
            "wg2s2_in": wg2s2,
        })
    return in_maps

